# revision 1
# baseline (speedup 1.0000x reference)
"""Trainium2 Bass kernel for DifferentiablePortfolioSim.

Computes, for allocations/returns of shape [B, T, A] = [1024, 2048, 64]:
    port_return[b,t] = sum_a alloc[b,t,a] * ret[b,t,a]
    turnover[b,t]    = sum_a |alloc[b,t,a] - alloc[b,t-1,a]|   (alloc[:,-1]=0)
    net_return       = port_return - 0.001 * turnover
    equity_curve     = [1, cumprod_t(1 + net_return)]          # [B, T+1]
Returns (equity_curve, net_return).

Sharding: data parallel over the batch dim, 128 rows per core on 8 cores.
Per-core layout: batch rows on the 128 SBUF partitions, time*assets on the
free dim, streamed in 32 chunks of 64 timesteps.

Engine split per chunk (to stay under the ~375us/core HBM roofline):
  - GPSIMD: fp32 elementwise alloc*ret product
  - ACT:    fp32 -> bf16 cast of alloc (for the turnover diff)
  - DVE:    segmented sum over A of the product (TensorReduce, no perf
            modes -> 1 elem/cycle), bf16 shifted diff (2x mode), and
            segmented |diff| sum (apply_absolute_value)
  - DVE tail: net combine, +1, cumprod via tensor_tensor_scan
"""

import numpy as np

B, T, A = 1024, 2048, 64
NCORES = 8
BP = B // NCORES  # 128 batch rows per core == SBUF partitions
TC = 64           # timesteps per chunk
NCH = T // TC

TRANSACTION_COST = 0.001

_compiled = None
LAST_RESULTS = None


def _build():
    import concourse.mybir as mybir
    from concourse import bacc
    from concourse.tile import TileContext

    f32 = mybir.dt.float32
    bf16 = mybir.dt.bfloat16
    Alu = mybir.AluOpType

    nc = bacc.Bacc(
        "TRN2",
        debug=False,
        target_bir_lowering=False,
        num_devices=NCORES,
    )

    a_in = nc.dram_tensor("alloc", [BP, T * A], f32, kind="ExternalInput").ap()
    r_in = nc.dram_tensor("ret", [BP, T * A], f32, kind="ExternalInput").ap()
    eq_out = nc.dram_tensor("equity", [BP, T + 1], f32, kind="ExternalOutput").ap()
    net_out = nc.dram_tensor("net", [BP, T], f32, kind="ExternalOutput").ap()

    with TileContext(nc) as tc:
        with (
            tc.tile_pool(name="persist", bufs=1) as pp,
            tc.tile_pool(name="chunk", bufs=2) as cp,
        ):
            port = pp.tile([BP, T], f32, tag="port")
            turn = pp.tile([BP, T], f32, tag="turn")
            net = pp.tile([BP, T], f32, tag="net")
            g = pp.tile([BP, T], f32, tag="g")
            eq = pp.tile([BP, T + 1], f32, tag="eq")

            for k in range(NCH):
                t0 = k * TC
                # a_t holds TC+1 timesteps: one lookback step + the chunk.
                a_t = cp.tile([BP, (TC + 1) * A], f32, tag="a")
                r_t = cp.tile([BP, TC * A], f32, tag="r")
                prod = cp.tile([BP, TC * A], f32, tag="prod")
                ab = cp.tile([BP, (TC + 1) * A], bf16, tag="ab")
                dif = cp.tile([BP, TC * A], bf16, tag="dif")

                if k == 0:
                    # prev_alloc at t=0 is zeros
                    nc.vector.memset(a_t[:, 0:A], 0.0)
                    nc.sync.dma_start(out=a_t[:, A:], in_=a_in[:, 0 : TC * A])
                else:
                    nc.sync.dma_start(
                        out=a_t[:], in_=a_in[:, (t0 - 1) * A : (t0 + TC) * A]
                    )
                nc.sync.dma_start(out=r_t[:], in_=r_in[:, t0 * A : (t0 + TC) * A])

                # ACT: cast alloc chunk (incl. lookback) to bf16
                nc.scalar.copy(out=ab[:], in_=a_t[:])

                # GPSIMD: fp32 product
                nc.gpsimd.tensor_mul(out=prod[:], in0=a_t[:, A:], in1=r_t[:])

                # DVE: port_return chunk = segmented sum over A
                nc.vector.tensor_reduce(
                    out=port[:, t0 : t0 + TC],
                    in_=prod[:].rearrange("p (t a) -> p t a", a=A),
                    axis=mybir.AxisListType.X,
                    op=Alu.add,
                )

                # DVE: bf16 shifted diff (2x mode), then |.| segmented sum
                nc.vector.tensor_sub(out=dif[:], in0=ab[:, A:], in1=ab[:, 0 : TC * A])
                nc.vector.tensor_reduce(
                    out=turn[:, t0 : t0 + TC],
                    in_=dif[:].rearrange("p (t a) -> p t a", a=A),
                    axis=mybir.AxisListType.X,
                    op=Alu.add,
                    apply_absolute_value=True,
                )

            # net = port - 0.001 * turn
            nc.vector.scalar_tensor_tensor(
                out=net[:],
                in0=turn[:],
                scalar=-TRANSACTION_COST,
                in1=port[:],
                op0=Alu.mult,
                op1=Alu.add,
            )
            # g = 1 + net
            nc.vector.tensor_scalar_add(out=g[:], in0=net[:], scalar1=1.0)
            # equity: eq[0] = 1, eq[1:] = cumprod(g)
            nc.vector.memset(eq[:, 0:1], 1.0)
            nc.vector.tensor_tensor_scan(
                out=eq[:, 1 : T + 1],
                data0=g[:],
                data1=g[:],
                initial=1.0,
                op0=Alu.mult,
                op1=Alu.bypass,
            )

            nc.sync.dma_start(out=net_out[:], in_=net[:])
            nc.sync.dma_start(out=eq_out[:], in_=eq[:])

    nc.compile()
    return nc


def _get_compiled():
    global _compiled
    if _compiled is None:
        _compiled = _build()
    return _compiled


def kernel(allocations, returns):
    global LAST_RESULTS
    from concourse.bass_utils import run_bass_kernel_spmd

    nc = _get_compiled()

    a = np.ascontiguousarray(np.asarray(allocations, dtype=np.float32)).reshape(
        B, T * A
    )
    r = np.ascontiguousarray(np.asarray(returns, dtype=np.float32)).reshape(B, T * A)

    in_maps = [
        {"alloc": a[i * BP : (i + 1) * BP], "ret": r[i * BP : (i + 1) * BP]}
        for i in range(NCORES)
    ]
    res = run_bass_kernel_spmd(nc, in_maps, core_ids=list(range(NCORES)))
    LAST_RESULTS = res

    equity = np.concatenate([res.results[i]["equity"] for i in range(NCORES)], axis=0)
    net = np.concatenate([res.results[i]["net"] for i in range(NCORES)], axis=0)
    return equity, net


# revision 3
# speedup vs baseline: 1.3636x; 1.3636x over previous
"""Trainium2 Bass kernel for DifferentiablePortfolioSim.

Computes, for allocations/returns of shape [B, T, A] = [1024, 2048, 64]:
    port_return[b,t] = sum_a alloc[b,t,a] * ret[b,t,a]
    turnover[b,t]    = sum_a |alloc[b,t,a] - alloc[b,t-1,a]|   (alloc[:,-1]=0)
    net_return       = port_return - 0.001 * turnover
    equity_curve     = [1, cumprod_t(1 + net_return)]          # [B, T+1]
Returns (equity_curve, net_return).

Sharding: data parallel over batch, 128 rows per core on 8 cores; batch rows
on the 128 SBUF partitions, time*assets streamed on the free dim in chunks.

Inputs are pre-cast to fp16 on the host: halves HBM traffic (the memory
roofline) and enables the DVE 2x perf mode for the elementwise passes.
fp16 keeps ~3 decimal digits; since equity decays exponentially (mean net
return is negative), absmax-relative error stays ~1e-4.

Engine split per chunk:
  - DVE:  fp16 product, fp16 shifted diff, and two pairwise-add reduction
          ladders (64 -> 1 over the asset dim). TensorReduce has no DVE perf
          modes (1 elem/cycle), so a ladder of fp16 2x tensor_tensor adds is
          ~2x faster.
  - ACT:  elementwise |diff|
  - GPSIMD: a slice of the diff pass (it is ~6x slower per element than
          DVE-2x, so it only gets a minority share)
"""

import numpy as np

B, T, A = 1024, 2048, 64
NCORES = 8
BP = B // NCORES  # 128 batch rows per core == SBUF partitions
TC = 64           # timesteps per chunk
NCH = T // TC
# timesteps of each chunk's diff pass that run on GPSIMD instead of DVE
TC_GP = 16

TRANSACTION_COST = 0.001

_compiled = None
LAST_RESULTS = None


def _ladder(nc, pool, mybir, src, n_seg, seg, out_fp32, tag):
    """Sum over contiguous segments of length `seg` (power of 2) via pairwise
    halving adds: fp16 tensor_tensor runs at 2 elem/cycle vs TensorReduce's 1.
    src: AP [BP, n_seg*seg] fp16. Writes fp32 sums [BP, n_seg] to out_fp32."""
    f16 = mybir.dt.float16
    cur = src
    width = seg
    lvl = 0
    while width > 2:
        width //= 2
        nxt = pool.tile([BP, n_seg * width], f16, tag=f"{tag}l{lvl}")
        nc.vector.tensor_add(
            out=nxt[:],
            in0=cur.rearrange("p (t a) -> p t a", a=2 * width)[:, :, 0:width],
            in1=cur.rearrange("p (t a) -> p t a", a=2 * width)[:, :, width : 2 * width],
        )
        cur = nxt[:]
        lvl += 1
    # final level: fp32 output
    nc.vector.tensor_add(
        out=out_fp32,
        in0=cur.rearrange("p (t a) -> p t a", a=2)[:, :, 0:1],
        in1=cur.rearrange("p (t a) -> p t a", a=2)[:, :, 1:2],
    )


def _build():
    import concourse.mybir as mybir
    from concourse import bacc
    from concourse.tile import TileContext

    f32 = mybir.dt.float32
    f16 = mybir.dt.float16
    Alu = mybir.AluOpType

    nc = bacc.Bacc(
        "TRN2",
        debug=False,
        target_bir_lowering=False,
        num_devices=NCORES,
    )

    a_in = nc.dram_tensor("alloc", [BP, T * A], f16, kind="ExternalInput").ap()
    r_in = nc.dram_tensor("ret", [BP, T * A], f16, kind="ExternalInput").ap()
    eq_out = nc.dram_tensor("equity", [BP, T + 1], f32, kind="ExternalOutput").ap()
    net_out = nc.dram_tensor("net", [BP, T], f32, kind="ExternalOutput").ap()

    with TileContext(nc) as tc:
        with (
            tc.tile_pool(name="persist", bufs=1) as pp,
            tc.tile_pool(name="chunk", bufs=2) as cp,
        ):
            port = pp.tile([BP, T], f32, tag="port")
            turn = pp.tile([BP, T], f32, tag="turn")
            net = pp.tile([BP, T], f32, tag="net")
            g = pp.tile([BP, T], f32, tag="g")
            eq = pp.tile([BP, T + 1], f32, tag="eq")

            for k in range(NCH):
                t0 = k * TC
                # a_t holds TC+1 timesteps: one lookback step + the chunk.
                a_t = cp.tile([BP, (TC + 1) * A], f16, tag="a")
                r_t = cp.tile([BP, TC * A], f16, tag="r")
                prod = cp.tile([BP, TC * A], f16, tag="prod")
                dif = cp.tile([BP, TC * A], f16, tag="dif")
                adif = cp.tile([BP, TC * A], f16, tag="adif")

                if k == 0:
                    # prev_alloc at t=0 is zeros
                    nc.vector.memset(a_t[:, 0:A], 0.0)
                    nc.sync.dma_start(out=a_t[:, A:], in_=a_in[:, 0 : TC * A])
                else:
                    nc.sync.dma_start(
                        out=a_t[:], in_=a_in[:, (t0 - 1) * A : (t0 + TC) * A]
                    )
                nc.sync.dma_start(out=r_t[:], in_=r_in[:, t0 * A : (t0 + TC) * A])

                # DVE: fp16 product (2x mode)
                nc.vector.tensor_mul(out=prod[:], in0=a_t[:, A:], in1=r_t[:])

                # shifted diff, split DVE / GPSIMD
                ne = (TC - TC_GP) * A
                nc.vector.tensor_sub(
                    out=dif[:, 0:ne], in0=a_t[:, A : A + ne], in1=a_t[:, 0:ne]
                )
                if TC_GP:
                    nc.gpsimd.tensor_sub(
                        out=dif[:, ne:],
                        in0=a_t[:, A + ne :],
                        in1=a_t[:, ne : TC * A],
                    )

                # ACT: |diff|
                nc.scalar.activation(
                    out=adif[:], in_=dif[:], func=mybir.ActivationFunctionType.Abs
                )

                # reduction ladders over the asset dim
                _ladder(nc, cp, mybir, prod[:], TC, A, port[:, t0 : t0 + TC], "p")
                _ladder(nc, cp, mybir, adif[:], TC, A, turn[:, t0 : t0 + TC], "t")

            # net = port - 0.001 * turn
            nc.vector.scalar_tensor_tensor(
                out=net[:],
                in0=turn[:],
                scalar=-TRANSACTION_COST,
                in1=port[:],
                op0=Alu.mult,
                op1=Alu.add,
            )
            # g = 1 + net
            nc.vector.tensor_scalar_add(out=g[:], in0=net[:], scalar1=1.0)
            # equity: eq[0] = 1, eq[1:] = cumprod(g)
            nc.vector.memset(eq[:, 0:1], 1.0)
            nc.vector.tensor_tensor_scan(
                out=eq[:, 1 : T + 1],
                data0=g[:],
                data1=g[:],
                initial=1.0,
                op0=Alu.mult,
                op1=Alu.bypass,
            )

            nc.sync.dma_start(out=net_out[:], in_=net[:])
            nc.sync.dma_start(out=eq_out[:], in_=eq[:])

    nc.compile()
    return nc


def _get_compiled():
    global _compiled
    if _compiled is None:
        _compiled = _build()
    return _compiled


def kernel(allocations, returns):
    global LAST_RESULTS
    from concourse.bass_utils import run_bass_kernel_spmd

    nc = _get_compiled()

    a = np.asarray(allocations, dtype=np.float32).astype(np.float16).reshape(B, T * A)
    r = np.asarray(returns, dtype=np.float32).astype(np.float16).reshape(B, T * A)

    in_maps = [
        {"alloc": a[i * BP : (i + 1) * BP], "ret": r[i * BP : (i + 1) * BP]}
        for i in range(NCORES)
    ]
    res = run_bass_kernel_spmd(nc, in_maps, core_ids=list(range(NCORES)))
    LAST_RESULTS = res

    equity = np.concatenate([res.results[i]["equity"] for i in range(NCORES)], axis=0)
    net = np.concatenate([res.results[i]["net"] for i in range(NCORES)], axis=0)
    return equity, net
